# Initial kernel scaffold
#
"""ComboSumModule kernel for Trainium2 (8 NeuronCores, SPMD data parallel).

outputs[i] = x_i.sum(axis=1) for six tensors x_i of shape (2048, L_i, C_i).
Sharding: pure data parallel over the leading batch axis (2048 -> 256/core).
Each core loads [128, L*C] contiguous tiles into SBUF and reduces over L on
the vector engine via a strided [128, C, L] access-pattern view.
"""

import numpy as np

import concourse.bass as bass
import concourse.tile as tile
from concourse import mybir
from concourse.bass_utils import run_bass_kernel_spmd

# (L, C) per tensor; batch axis is 2048 split 8 ways.
SHAPES = [(128, 64), (32, 32), (64, 64), (16, 48), (200, 32), (8, 8)]
B = 2048
NCORES = 8
BLOC = B // NCORES  # 256 batch rows per core
P = 128             # SBUF partitions

_NC_CACHE = None


def _build_bass():
    nc = bass.Bass()
    f32 = mybir.dt.float32
    xs = [
        nc.declare_dram_parameter(f"x{i}", [BLOC, L * C], f32, isOutput=False)
        for i, (L, C) in enumerate(SHAPES)
    ]
    ys = [
        nc.declare_dram_parameter(f"y{i}", [BLOC, C], f32, isOutput=True)
        for i, (L, C) in enumerate(SHAPES)
    ]

    with tile.TileContext(nc) as tc:
        with tc.tile_pool(name="in", bufs=4) as pin, tc.tile_pool(
            name="out", bufs=4
        ) as pout:
            for i, (L, C) in enumerate(SHAPES):
                for pb in range(BLOC // P):
                    t = pin.tile([P, L * C], f32, tag="in")
                    nc.sync.dma_start(
                        out=t[:], in_=xs[i][pb * P : (pb + 1) * P, :]
                    )
                    o = pout.tile([P, C], f32, tag="out")
                    nc.vector.tensor_reduce(
                        out=o[:],
                        in_=t[:].rearrange("p (l c) -> p c l", c=C),
                        axis=mybir.AxisListType.X,
                        op=mybir.AluOpType.add,
                    )
                    nc.sync.dma_start(
                        out=ys[i][pb * P : (pb + 1) * P, :], in_=o[:]
                    )
    return nc


def _get_nc():
    global _NC_CACHE
    if _NC_CACHE is None:
        _NC_CACHE = _build_bass()
    return _NC_CACHE


def _run(inputs: dict, **spmd_kwargs):
    dim = int(np.asarray(inputs["dim"]))
    assert dim == 1, f"kernel hardcodes reduction over axis 1, got {dim}"
    xs = [
        np.ascontiguousarray(np.asarray(inputs[f"x{i}"], dtype=np.float32))
        for i in range(6)
    ]
    in_maps = []
    for c in range(NCORES):
        m = {}
        for i, (L, C) in enumerate(SHAPES):
            m[f"x{i}"] = np.ascontiguousarray(
                xs[i][c * BLOC : (c + 1) * BLOC].reshape(BLOC, L * C)
            )
        in_maps.append(m)
    br = run_bass_kernel_spmd(
        _get_nc(), in_maps, core_ids=list(range(NCORES)), **spmd_kwargs
    )
    outs = tuple(
        np.concatenate([br.results[c][f"y{i}"] for c in range(NCORES)], axis=0)
        for i in range(6)
    )
    return outs, br


def kernel(**inputs):
    outs, _ = _run(inputs)
    return outs


# revision 7
# speedup vs baseline: 2.1652x; 2.1652x over previous
"""ComboSumModule kernel for Trainium2 (8 NeuronCores, SPMD data parallel).

outputs[i] = x_i.sum(axis=1) for six tensors x_i of shape (2048, L_i, C_i).
Sharding: pure data parallel over the leading batch axis (2048 -> 256/core).
Each core loads [128, L*C] contiguous tiles into SBUF and reduces over L on
the vector engine via a strided [128, C, L] access-pattern view. All six
per-tensor results for a 128-row block are packed into one [128, 248] tile
and stored with a single DMA (the HWDGE DMA instruction format only allows
one sync wait, so the kernel is structured to need at most one per op).
"""

import numpy as np

import concourse.bass as bass
import concourse.tile as tile
from concourse import mybir
from concourse.bass_utils import run_bass_kernel_spmd

# (L, C) per tensor; batch axis is 2048 split 8 ways.
SHAPES = [(128, 64), (32, 32), (64, 64), (16, 48), (200, 32), (8, 8)]
CSUM = int(np.sum([c for _, c in SHAPES]))  # 248 packed output columns
OFFS = np.concatenate([[0], np.cumsum([c for _, c in SHAPES])]).astype(int)
B = 2048
NCORES = 8
BLOC = B // NCORES  # 256 batch rows per core
P = 128             # SBUF partitions

_NC_CACHE = None


def _build_bass(reps: int = 1):
    nc = bass.Bass()
    f32 = mybir.dt.float32
    xs = [
        nc.declare_dram_parameter(f"x{i}", [BLOC, L * C], f32, isOutput=False)
        for i, (L, C) in enumerate(SHAPES)
    ]
    y = nc.declare_dram_parameter("y", [BLOC, CSUM], f32, isOutput=True)

    with tile.TileContext(nc) as tc:
        for rep in range(reps):
            with tc.tile_pool(name=f"in{rep}", bufs=1) as pin, tc.tile_pool(
                name=f"out{rep}", bufs=1
            ) as pout:
                for pb in range(BLOC // P):
                    rows = slice(pb * P, (pb + 1) * P)
                    ot = pout.tile([P, CSUM], f32, tag=f"out{pb}")
                    for i, (L, C) in enumerate(SHAPES):
                        t = pin.tile([P, L * C], f32, tag=f"in{pb}_{i}")
                        nc.sync.dma_start(out=t[:], in_=xs[i][rows, :])
                        nc.vector.tensor_reduce(
                            out=ot[:, int(OFFS[i]) : int(OFFS[i + 1])],
                            in_=t[:].rearrange("p (l c) -> p c l", c=C),
                            axis=mybir.AxisListType.X,
                            op=mybir.AluOpType.add,
                        )
                    nc.gpsimd.dma_start(out=y[rows, :], in_=ot[:])
    return nc


def _legalize_sync_waits(nc, max_waits: int = 1):
    """The TRN2 instruction encodings hold at most one sync-wait. Tile can
    attach several (notably the kernel-tail drain waits on every semaphore);
    hoist the excess onto single-wait EventSemaphore ops inserted just before
    the offending instruction on the same engine."""
    n_new = 0
    for fn in nc.m.functions:
        for blk in fn.blocks:
            insts = blk.instructions
            i = 0
            while i < len(insts):
                inst = insts[i]
                si = inst.sync_info
                waits = list(si.on_wait) if si is not None and si.on_wait else []
                if len(waits) > max_waits:
                    hoist, keep = waits[:-max_waits], waits[-max_waits:]
                    for k, w in enumerate(hoist):
                        ev = mybir.InstEventSemaphore(
                            name=f"{inst.name}-hw{n_new}"
                        )
                        n_new += 1
                        ev.engine = inst.engine
                        ev.sync_info = mybir.SyncInfo(on_wait=[w], on_update=[])
                        insts.insert(i + k, ev)
                    i += len(hoist)
                    inst.sync_info = mybir.SyncInfo(
                        on_wait=keep, on_update=list(si.on_update or [])
                    )
                i += 1
    return nc


def _get_nc():
    global _NC_CACHE
    if _NC_CACHE is None:
        _NC_CACHE = _legalize_sync_waits(_build_bass())
    return _NC_CACHE


def _run(inputs: dict, **spmd_kwargs):
    dim = int(np.asarray(inputs["dim"]))
    assert dim == 1, f"kernel hardcodes reduction over axis 1, got {dim}"
    xs = [
        np.ascontiguousarray(np.asarray(inputs[f"x{i}"], dtype=np.float32))
        for i in range(6)
    ]
    in_maps = []
    for c in range(NCORES):
        m = {}
        for i, (L, C) in enumerate(SHAPES):
            m[f"x{i}"] = np.ascontiguousarray(
                xs[i][c * BLOC : (c + 1) * BLOC].reshape(BLOC, L * C)
            )
        in_maps.append(m)
    br = run_bass_kernel_spmd(
        _get_nc(), in_maps, core_ids=list(range(NCORES)), **spmd_kwargs
    )
    packed = np.concatenate([br.results[c]["y"] for c in range(NCORES)], axis=0)
    outs = tuple(
        np.ascontiguousarray(packed[:, int(OFFS[i]) : int(OFFS[i + 1])])
        for i in range(6)
    )
    return outs, br


def kernel(**inputs):
    outs, _ = _run(inputs)
    return outs
